# revision 26
# baseline (speedup 1.0000x reference)
"""Trainium2 Bass kernel for multi-head attention (B=4, NQ=NK=2048, E=1024, H=16).

Sharding: 8 cores = 4 batches x 2 head-groups (8 heads each).
Each core computes its head-group's attention and a partial output
projection; the host sums the two partials per batch and adds bo.

Per-core layouts:
  qhT/khT: [128 part (pair of heads, 64 each), pair, seq] (head-dim on
           partitions) - produced by PE-transposing the inputs then
           projecting with the weight as the stationary operand.
  vh:      [128 part (k), kblock, head, 65] bf16; col 64 is ones so the AV
           matmul's output row 64 is the softmax denominator.
  scores:  [k, q] (transposed) so the AV lhsT is the exp tile's natural
           layout and the denominator reduction rides the AV matmul.
QK^T runs as two 64x128 row-tiles (tile_position (0,0)/(64,0)) per head
pair, writing one [128, 1024] PSUM tile that a single Exp instruction
evacuates. The output projection is interleaved per 512-q chunk.
"""

import math
from contextlib import ExitStack

import numpy as np

import concourse.bass as bass
import concourse.mybir as mybir
import concourse.tile as tile
from concourse.vector_clock import ScopedClock

F32R = mybir.dt.float32r
F32 = mybir.dt.float32
BF16 = mybir.dt.bfloat16

B = 4
NQ = 2048
NK = 2048
E = 1024
H = 16
HD = 64
INT = 1024
IL = INT // 2          # per-core internal dim = 512
HL = H // 2            # heads per core = 8
N_CORES = 8


class _SplitDrainTC(tile.TileContext):
    """TileContext whose tail drain chains single-wait Drain instructions.

    The walrus build here rejects Drain instructions carrying more than one
    semaphore wait ("Too many sync wait commands"), while the stock Tile
    tail-drain waits on the whole vector clock in one instruction.
    """

    def _drain_and_barrier(self, tick_clock, wait_clock):
        drain_inst = self.nc.sync.drain()
        wait_clock.add_sem_waits(
            drain_inst.ins, ScopedClock({None: tick_clock.global_clock})
        )
        si = drain_inst.ins.sync_info
        waits = list(si.on_wait) if si and si.on_wait else []
        if len(waits) > 1:
            drain_inst.ins.sync_info = mybir.SyncInfo(
                on_wait=waits[:1], on_update=list(si.on_update or [])
            )
            for i in range(1, len(waits)):
                extra = self.nc.sync.drain()
                extra.ins.sync_info = mybir.SyncInfo(
                    on_wait=waits[i : i + 1], on_update=[]
                )
        self.nc.all_engine_barrier()
        assert self.sems is not None
        popped = self.nc._tile_sem_poison_stack.pop()
        assert popped is self._sem_poison
        self.nc.clear_and_free_semaphores(list(self.sems.allocated().values()))
        self.nc.all_engine_barrier()


def _split_waits(nc, maxw=1):
    """Hoist excess semaphore waits onto same-engine NoOps.

    This walrus build rejects instructions carrying more than one sem wait
    ("Too many sync wait commands"), while Tile attaches the full required
    wait set to each instruction. Same-engine program order makes the
    preceding NoOp waits equivalent.
    """
    for fn in nc.m.functions:
        for blk in fn.blocks:
            insts = list(blk.instructions)
            out = []
            changed = False
            for inst in insts:
                si = inst.sync_info
                waits = list(si.on_wait) if si and si.on_wait else []
                if len(waits) > maxw:
                    changed = True
                    extra, keep = waits[:-maxw], waits[-maxw:]
                    for w in extra:
                        out.append(
                            mybir.InstNoOp(
                                name=nc.get_next_instruction_name(),
                                ins=[],
                                outs=[],
                                engine=inst.engine,
                                sync_info=mybir.SyncInfo(
                                    on_wait=[w], on_update=[]
                                ),
                                bass_nofuse=True,
                            )
                        )
                    inst.sync_info = mybir.SyncInfo(
                        on_wait=keep, on_update=list(si.on_update or [])
                    )
                out.append(inst)
            if changed:
                blk.instructions = out


def build_nc(
    nq=NQ,
    nk=NK,
    split_waits=True,
    nat_bufs=6,
    xt_bufs=12,
    exps_bufs=22,
    sp_bufs=2,
    av_bufs=2,
    op_bufs=1,
    pp_bufs=5,
    tp_bufs=3,
):
    """Build the per-core Bass module (SPMD; all cores run this program)."""
    nc = bass.Bass()

    q_h = nc.declare_dram_parameter("q", [nq, E], F32R, isOutput=False)
    k_h = nc.declare_dram_parameter("k", [nk, E], F32R, isOutput=False)
    v_h = nc.declare_dram_parameter("v", [nk, E], F32R, isOutput=False)
    wq_h = nc.declare_dram_parameter("wq", [E, IL], F32R, isOutput=False)
    wk_h = nc.declare_dram_parameter("wk", [E, IL], F32R, isOutput=False)
    wv_h = nc.declare_dram_parameter("wv", [E, IL], F32R, isOutput=False)
    bq_h = nc.declare_dram_parameter("bq", [IL], F32, isOutput=False)
    bk_h = nc.declare_dram_parameter("bk", [IL], F32, isOutput=False)
    bv_h = nc.declare_dram_parameter("bv", [IL], F32, isOutput=False)
    wo_h = nc.declare_dram_parameter("wo", [IL, E], F32R, isOutput=False)
    id_h = nc.declare_dram_parameter("ident", [128, 128], F32R, isOutput=False)
    out_h = nc.declare_dram_parameter("out", [nq, E], F32R, isOutput=True)

    EC = E // 128        # 8 E-chunks
    IB = IL // 128       # 4 INT-blocks (= head pairs)
    KB = nk // 128       # k blocks
    SCALE = 1.0 / math.sqrt(HD)

    with _SplitDrainTC(nc) as tc, ExitStack() as top:
        singles = top.enter_context(tc.tile_pool(name="singles", bufs=1))
        persist = top.enter_context(tc.tile_pool(name="persist", bufs=1))

        identity = singles.tile([128, 128], F32R)
        nc.sync.dma_start(out=identity, in_=id_h.ap())
        # biases as [128 part (INT within block), block] per-partition columns
        bq_sb = singles.tile([128, IB], F32)
        bk_sb = singles.tile([128, IB], F32)
        bv_sb = singles.tile([128, IB], F32)
        for b_sb, b_h in ((bq_sb, bq_h), (bk_sb, bk_h), (bv_sb, bv_h)):
            nc.sync.dma_start(
                out=b_sb, in_=b_h.ap().rearrange("(c p) -> p c", p=128)
            )

        qhT = persist.tile([128, IB, nq], F32R)
        khT = persist.tile([128, IB, nk], F32R)
        vh = persist.tile([128, KB, HL, 65], BF16)
        nc.vector.memset(vh[:, :, :, 64:65], 1.0)
        if "B" not in phases:
            nc.vector.memset(qhT, 0.5)
            nc.vector.memset(khT, 0.5)
            nc.vector.memset(vh[:, :, :, 0:64], 0.5)

        # ---------------- Phase B: transpose inputs + projections ----------
        with ExitStack() as phb:
            wpool = phb.enter_context(tc.tile_pool(name="weights", bufs=1))
            nat_p = phb.enter_context(tc.tile_pool(name="nat", bufs=nat_bufs))
            xt_p = phb.enter_context(tc.tile_pool(name="xT", bufs=xt_bufs))
            tpsum = phb.enter_context(
                tc.tile_pool(name="tpsum", bufs=tp_bufs, space="PSUM")
            )
            ppsum = phb.enter_context(
                tc.tile_pool(name="ppsum", bufs=pp_bufs, space="PSUM")
            )

            wq_sb = wpool.tile([128, EC, IL], F32R)
            wk_sb = wpool.tile([128, EC, IL], F32R)
            wv_sb = wpool.tile([128, EC, IL], F32R)
            for w_sb, w_h in ((wq_sb, wq_h), (wk_sb, wk_h), (wv_sb, wv_h)):
                nc.sync.dma_start(
                    out=w_sb, in_=w_h.ap().rearrange("(c p) i -> p c i", p=128)
                )

            def load_and_transpose(src_h, sc):
                """Load rows [sc*512, sc*512+512) of src; return 8 xT tiles
                [128 E-part, 512 seq]."""
                nats = []
                for sb in range(4):
                    nat = nat_p.tile([128, E], F32R, tag="nat")
                    r0 = sc * 512 + sb * 128
                    nc.sync.dma_start(out=nat, in_=src_h.ap()[r0 : r0 + 128, :])
                    nats.append(nat)
                xts = []
                for ec in range(EC):
                    ps = tpsum.tile([128, 512], F32R, tag="t")
                    for sb in range(4):
                        nc.tensor.transpose(
                            ps[:, sb * 128 : sb * 128 + 128],
                            nats[sb][:, ec * 128 : ec * 128 + 128],
                            identity,
                        )
                    xt = xt_p.tile([128, 512], F32R, tag="xt")
                    nc.vector.tensor_copy(out=xt, in_=ps)
                    xts.append(xt)
                return xts

            def project_qk(dst, src_h, w_sb, b_sb, sc):
                """One 512-seq chunk of the q/k projection into dst (+bias)."""
                xts = load_and_transpose(src_h, sc)
                for ib in range(IB):
                    ps = ppsum.tile([128, 512], F32, tag="p")
                    for ec in range(EC):
                        nc.tensor.matmul(
                            ps,
                            w_sb[:, ec, ib * 128 : ib * 128 + 128],
                            xts[ec],
                            start=(ec == 0),
                            stop=(ec == EC - 1),
                        )
                    nc.vector.tensor_scalar_add(
                        out=dst[:, ib, sc * 512 : sc * 512 + 512],
                        in0=ps,
                        scalar1=b_sb[:, ib : ib + 1],
                    )

            def project_v(sc):
                """One 512-seq chunk of the v projection into vh (bf16)."""
                xts = load_and_transpose(v_h, sc)
                for sb in range(4):
                    ps = ppsum.tile([128, 512], F32, tag="p")
                    for ec in range(EC):
                        nc.tensor.matmul(
                            ps,
                            xts[ec][:, sb * 128 : sb * 128 + 128],
                            wv_sb[:, ec, :],
                            start=(ec == 0),
                            stop=(ec == EC - 1),
                        )
                    kb = sc * 4 + sb
                    nc.vector.tensor_copy(
                        out=vh[:, kb, :, 0:64],
                        in_=ps.rearrange("p (h d) -> p h d", h=HL),
                    )

            # k and v first so attention can start as soon as the first q
            # chunk lands; v-projection bias is applied post-normalization
            # on attnT.
            for sc in range(nk // 512):
                project_qk(khT, k_h, wk_sb, bk_sb, sc)
            for sc in range(nk // 512):
                project_v(sc)
            for sc in range(nq // 512):
                project_qk(qhT, q_h, wq_sb, bq_sb, sc)

        # -------- Phase C+D: attention with interleaved output projection ---
        with ExitStack() as phc:
            cd_pool = phc.enter_context(tc.tile_pool(name="cd", bufs=1))
            wo_sb = cd_pool.tile([128, IB, E], F32R)
            nc.sync.dma_start(
                out=wo_sb, in_=wo_h.ap().rearrange("(c p) e -> p c e", p=128)
            )
            attnT = cd_pool.tile([128, IB, nq], F32R)

            exps_p = phc.enter_context(tc.tile_pool(name="exps", bufs=exps_bufs))
            rl_p = phc.enter_context(tc.tile_pool(name="rl", bufs=4))
            rld_p = phc.enter_context(
                tc.tile_pool(name="rld", bufs=8, space="DRAM")
            )
            out_p = phc.enter_context(tc.tile_pool(name="outsb", bufs=4))
            spsum = phc.enter_context(
                tc.tile_pool(name="spsum", bufs=sp_bufs, space="PSUM")
            )
            avpsum = phc.enter_context(
                tc.tile_pool(name="avpsum", bufs=av_bufs, space="PSUM")
            )
            opsum = phc.enter_context(
                tc.tile_pool(name="opsum", bufs=op_bufs, space="PSUM")
            )

            for jq in range(nq // 512):
                q0 = jq * 512
                for p_ in range(IB):
                    # QK^T for both heads of the pair: h2=0 scores in psum
                    # cols 0:512 (bank A), h2=1 in cols 512:1024 (bank B) -
                    # independent 64x128 row-tiles that overlap on the PE.
                    exp_tiles = []
                    for kb in range(KB):
                        ps = spsum.tile([128, 1024], F32, tag="s")
                        for h2 in range(2):
                            hp = slice(h2 * 64, h2 * 64 + 64)
                            nc.tensor.matmul(
                                ps[:, h2 * 512 : h2 * 512 + 512],
                                khT[hp, p_, kb * 128 : kb * 128 + 128],
                                qhT[hp, p_, q0 : q0 + 512],
                                start=True,
                                stop=True,
                            )
                        et = exps_p.tile([128, 1024], BF16, tag="e")
                        nc.scalar.activation(
                            out=et,
                            in_=ps,
                            func=mybir.ActivationFunctionType.Exp,
                            scale=SCALE,
                        )
                        exp_tiles.append(et)
                    # AV (ones-augmented): rows 0..63 data, row 64 = denom
                    for h2 in range(2):
                        h = 2 * p_ + h2
                        ps = avpsum.tile([65, 512], F32, tag="av")
                        for kb in range(KB):
                            nc.tensor.matmul(
                                ps,
                                vh[:, kb, h, :],
                                exp_tiles[kb][:, h2 * 512 : h2 * 512 + 512],
                                start=(kb == 0),
                                stop=(kb == KB - 1),
                            )
                        # 1/l: reciprocal of the denom row, bounce through
                        # DRAM, DMA back with a stride-0 partition AP
                        rl1 = rl_p.tile([1, 512], F32, tag="rl1")
                        nc.vector.reciprocal(out=rl1, in_=ps[64:65, :])
                        rld = rld_p.tile([1, 512], F32, tag="rld")
                        nc.sync.dma_start(out=rld, in_=rl1)
                        rlb = rl_p.tile([64, 512], F32, tag="rlb")
                        nc.sync.dma_start(
                            out=rlb,
                            in_=bass.AP(
                                tensor=rld.tensor,
                                offset=rld.offset,
                                ap=[[0, 64]] + list(rld.ap)[1:],
                            ),
                        )
                        hrows = slice(0, 64) if h2 == 0 else slice(64, 128)
                        nc.vector.tensor_mul(
                            attnT[hrows, p_, q0 : q0 + 512],
                            ps[0:64, :],
                            rlb,
                        )
                    # + bv (deferred v-projection bias: P @ (vh + bv) =
                    #   P@vh + l*bv, so attnT just gains bv after normalize)
                    nc.vector.tensor_scalar_add(
                        out=attnT[:, p_, q0 : q0 + 512],
                        in0=attnT[:, p_, q0 : q0 + 512],
                        scalar1=bv_sb[:, p_ : p_ + 1],
                    )

                # output projection for this 512-q chunk
                for m2 in range(4):
                    m = jq * 4 + m2
                    for half in range(2):
                        ps = opsum.tile([128, 512], F32, tag="o")
                        for ic in range(IB):
                            nc.tensor.matmul(
                                ps,
                                attnT[:, ic, m * 128 : m * 128 + 128],
                                wo_sb[:, ic, half * 512 : half * 512 + 512],
                                start=(ic == 0),
                                stop=(ic == IB - 1),
                            )
                        osb = out_p.tile([128, 512], F32R, tag="osb")
                        nc.vector.tensor_copy(out=osb, in_=ps)
                        nc.sync.dma_start(
                            out=out_h.ap()[
                                m * 128 : m * 128 + 128,
                                half * 512 : half * 512 + 512,
                            ],
                            in_=osb,
                        )

    if split_waits:
        _split_waits(nc)
    return nc


_CACHED = {}


def _get_nc(nq=NQ, nk=NK):
    key = (nq, nk)
    if key not in _CACHED:
        _CACHED[key] = build_nc(nq, nk)
    return _CACHED[key]


def shard_inputs(q, k, v, wq, bq, wk, bk, wv, bv, wo):
    """8 per-core input maps: core c -> (batch c//2, head-group c%2)."""
    in_maps = []
    for c in range(N_CORES):
        b, g = c // 2, c % 2
        sl = slice(g * IL, (g + 1) * IL)
        in_maps.append(
            {
                "q": np.ascontiguousarray(q[b]),
                "k": np.ascontiguousarray(k[b]),
                "v": np.ascontiguousarray(v[b]),
                "wq": np.ascontiguousarray(wq[:, sl]),
                "wk": np.ascontiguousarray(wk[:, sl]),
                "wv": np.ascontiguousarray(wv[:, sl]),
                "bq": np.ascontiguousarray(bq[sl]),
                "bk": np.ascontiguousarray(bk[sl]),
                "bv": np.ascontiguousarray(bv[sl]),
                "wo": np.ascontiguousarray(wo[sl, :]),
                "ident": np.eye(128, dtype=np.float32),
            }
        )
    return in_maps


def kernel(q, k, v, wq, bq, wk, bk, wv, bv, wo, bo, _trace=False):
    from concourse.bass_utils import run_bass_kernel_spmd

    q, k, v = (np.asarray(x, np.float32) for x in (q, k, v))
    wq, bq, wk, bk, wv, bv, wo, bo = (
        np.asarray(x, np.float32) for x in (wq, bq, wk, bk, wv, bv, wo, bo)
    )
    nc = _get_nc()
    in_maps = shard_inputs(q, k, v, wq, bq, wk, bk, wv, bv, wo)
    try:
        res = run_bass_kernel_spmd(
            nc, in_maps, core_ids=list(range(N_CORES)), trace=_trace
        )
    except Exception:
        if not _trace:
            raise
        import traceback

        traceback.print_exc()
        print("trace run failed; retrying without trace", flush=True)
        res = run_bass_kernel_spmd(
            nc, in_maps, core_ids=list(range(N_CORES)), trace=False
        )
    parts = np.stack([res.results[c]["out"] for c in range(N_CORES)])
    out = parts.reshape(B, 2, NQ, E).sum(axis=1) + bo[None, None, :]
    if _trace:
        kernel.last_results = res
    return out.astype(np.float32)


# revision 27
# speedup vs baseline: 1.1570x; 1.1570x over previous
"""Trainium2 Bass kernel for multi-head attention (B=4, NQ=NK=2048, E=1024, H=16).

Sharding: 8 cores = 4 batches x 2 head-groups (8 heads each).
Each core computes its head-group's attention and a partial output
projection; the host sums the two partials per batch and adds bo.

Per-core layouts:
  qhT/khT: [128 part (pair of heads, 64 each), pair, seq] (head-dim on
           partitions) - produced by PE-transposing the inputs then
           projecting with the weight as the stationary operand.
  vh:      [128 part (k), kblock, head, 65] bf16; col 64 is ones so the AV
           matmul's output row 64 is the softmax denominator.
  scores:  [k, q] (transposed) so the AV lhsT is the exp tile's natural
           layout and the denominator reduction rides the AV matmul.
QK^T runs as two 64x128 row-tiles (tile_position (0,0)/(64,0)) per head
pair, writing one [128, 1024] PSUM tile that a single Exp instruction
evacuates. The output projection is interleaved per 512-q chunk.
"""

import math
from contextlib import ExitStack

import numpy as np

import concourse.bass as bass
import concourse.mybir as mybir
import concourse.tile as tile
from concourse.vector_clock import ScopedClock

F32R = mybir.dt.float32r
F32 = mybir.dt.float32
BF16 = mybir.dt.bfloat16

B = 4
NQ = 2048
NK = 2048
E = 1024
H = 16
HD = 64
INT = 1024
IL = INT // 2          # per-core internal dim = 512
HL = H // 2            # heads per core = 8
N_CORES = 8


class _SplitDrainTC(tile.TileContext):
    """TileContext whose tail drain chains single-wait Drain instructions.

    The walrus build here rejects Drain instructions carrying more than one
    semaphore wait ("Too many sync wait commands"), while the stock Tile
    tail-drain waits on the whole vector clock in one instruction.
    """

    def _drain_and_barrier(self, tick_clock, wait_clock):
        drain_inst = self.nc.sync.drain()
        wait_clock.add_sem_waits(
            drain_inst.ins, ScopedClock({None: tick_clock.global_clock})
        )
        si = drain_inst.ins.sync_info
        waits = list(si.on_wait) if si and si.on_wait else []
        if len(waits) > 1:
            drain_inst.ins.sync_info = mybir.SyncInfo(
                on_wait=waits[:1], on_update=list(si.on_update or [])
            )
            for i in range(1, len(waits)):
                extra = self.nc.sync.drain()
                extra.ins.sync_info = mybir.SyncInfo(
                    on_wait=waits[i : i + 1], on_update=[]
                )
        self.nc.all_engine_barrier()
        assert self.sems is not None
        popped = self.nc._tile_sem_poison_stack.pop()
        assert popped is self._sem_poison
        self.nc.clear_and_free_semaphores(list(self.sems.allocated().values()))
        self.nc.all_engine_barrier()


def _split_waits(nc, maxw=1):
    """Hoist excess semaphore waits onto same-engine NoOps.

    This walrus build rejects instructions carrying more than one sem wait
    ("Too many sync wait commands"), while Tile attaches the full required
    wait set to each instruction. Same-engine program order makes the
    preceding NoOp waits equivalent.
    """
    for fn in nc.m.functions:
        for blk in fn.blocks:
            insts = list(blk.instructions)
            out = []
            changed = False
            for inst in insts:
                si = inst.sync_info
                waits = list(si.on_wait) if si and si.on_wait else []
                if len(waits) > maxw:
                    changed = True
                    extra, keep = waits[:-maxw], waits[-maxw:]
                    for w in extra:
                        out.append(
                            mybir.InstNoOp(
                                name=nc.get_next_instruction_name(),
                                ins=[],
                                outs=[],
                                engine=inst.engine,
                                sync_info=mybir.SyncInfo(
                                    on_wait=[w], on_update=[]
                                ),
                                bass_nofuse=True,
                            )
                        )
                    inst.sync_info = mybir.SyncInfo(
                        on_wait=keep, on_update=list(si.on_update or [])
                    )
                out.append(inst)
            if changed:
                blk.instructions = out


def build_nc(
    nq=NQ,
    nk=NK,
    split_waits=True,
    nat_bufs=10,
    xt_bufs=12,
    exps_bufs=22,
    sp_bufs=2,
    av_bufs=2,
    op_bufs=1,
    pp_bufs=4,
    tp_bufs=4,
):
    """Build the per-core Bass module (SPMD; all cores run this program)."""
    nc = bass.Bass()

    q_h = nc.declare_dram_parameter("q", [nq, E], F32R, isOutput=False)
    k_h = nc.declare_dram_parameter("k", [nk, E], F32R, isOutput=False)
    v_h = nc.declare_dram_parameter("v", [nk, E], F32R, isOutput=False)
    wq_h = nc.declare_dram_parameter("wq", [E, IL], F32R, isOutput=False)
    wk_h = nc.declare_dram_parameter("wk", [E, IL], F32R, isOutput=False)
    wv_h = nc.declare_dram_parameter("wv", [E, IL], F32R, isOutput=False)
    bq_h = nc.declare_dram_parameter("bq", [IL], F32, isOutput=False)
    bk_h = nc.declare_dram_parameter("bk", [IL], F32, isOutput=False)
    bv_h = nc.declare_dram_parameter("bv", [IL], F32, isOutput=False)
    wo_h = nc.declare_dram_parameter("wo", [IL, E], F32R, isOutput=False)
    id_h = nc.declare_dram_parameter("ident", [128, 128], F32R, isOutput=False)
    out_h = nc.declare_dram_parameter("out", [nq, E], F32R, isOutput=True)

    EC = E // 128        # 8 E-chunks
    IB = IL // 128       # 4 INT-blocks (= head pairs)
    KB = nk // 128       # k blocks
    SCALE = 1.0 / math.sqrt(HD)

    with _SplitDrainTC(nc) as tc, ExitStack() as top:
        singles = top.enter_context(tc.tile_pool(name="singles", bufs=1))
        persist = top.enter_context(tc.tile_pool(name="persist", bufs=1))

        identity = singles.tile([128, 128], F32R)
        nc.sync.dma_start(out=identity, in_=id_h.ap())
        # biases as [128 part (INT within block), block] per-partition columns
        bq_sb = singles.tile([128, IB], F32)
        bk_sb = singles.tile([128, IB], F32)
        bv_sb = singles.tile([128, IB], F32)
        for b_sb, b_h in ((bq_sb, bq_h), (bk_sb, bk_h), (bv_sb, bv_h)):
            nc.sync.dma_start(
                out=b_sb, in_=b_h.ap().rearrange("(c p) -> p c", p=128)
            )

        qhT = persist.tile([128, IB, nq], F32R)
        khT = persist.tile([128, IB, nk], F32R)
        vh = persist.tile([128, KB, HL, 65], BF16)
        nc.vector.memset(vh[:, :, :, 64:65], 1.0)
        if "B" not in phases:
            nc.vector.memset(qhT, 0.5)
            nc.vector.memset(khT, 0.5)
            nc.vector.memset(vh[:, :, :, 0:64], 0.5)

        # ---------------- Phase B: transpose inputs + projections ----------
        with ExitStack() as phb:
            wpool = phb.enter_context(tc.tile_pool(name="weights", bufs=1))
            nat_p = phb.enter_context(tc.tile_pool(name="nat", bufs=nat_bufs))
            xt_p = phb.enter_context(tc.tile_pool(name="xT", bufs=xt_bufs))
            tpsum = phb.enter_context(
                tc.tile_pool(name="tpsum", bufs=tp_bufs, space="PSUM")
            )
            ppsum = phb.enter_context(
                tc.tile_pool(name="ppsum", bufs=pp_bufs, space="PSUM")
            )

            wq_sb = wpool.tile([128, EC, IL], F32R)
            wk_sb = wpool.tile([128, EC, IL], F32R)
            wv_sb = wpool.tile([128, EC, IL], F32R)
            for w_sb, w_h in ((wq_sb, wq_h), (wk_sb, wk_h), (wv_sb, wv_h)):
                nc.sync.dma_start(
                    out=w_sb, in_=w_h.ap().rearrange("(c p) i -> p c i", p=128)
                )

            def load_and_transpose(src_h, sc):
                """Load rows [sc*512, sc*512+512) of src; return 8 xT tiles
                [128 E-part, 512 seq]."""
                nats = []
                for sb in range(4):
                    nat = nat_p.tile([128, E], F32R, tag="nat")
                    r0 = sc * 512 + sb * 128
                    nc.sync.dma_start(out=nat, in_=src_h.ap()[r0 : r0 + 128, :])
                    nats.append(nat)
                xts = []
                for ec in range(EC):
                    ps = tpsum.tile([128, 512], F32R, tag="t")
                    for sb in range(4):
                        nc.tensor.transpose(
                            ps[:, sb * 128 : sb * 128 + 128],
                            nats[sb][:, ec * 128 : ec * 128 + 128],
                            identity,
                        )
                    xt = xt_p.tile([128, 512], F32R, tag="xt")
                    nc.vector.tensor_copy(out=xt, in_=ps)
                    xts.append(xt)
                return xts

            def project_qk(dst, src_h, w_sb, b_sb, sc):
                """One 512-seq chunk of the q/k projection into dst (+bias)."""
                xts = load_and_transpose(src_h, sc)
                for ib in range(IB):
                    ps = ppsum.tile([128, 512], F32, tag="p")
                    for ec in range(EC):
                        nc.tensor.matmul(
                            ps,
                            w_sb[:, ec, ib * 128 : ib * 128 + 128],
                            xts[ec],
                            start=(ec == 0),
                            stop=(ec == EC - 1),
                        )
                    nc.vector.tensor_scalar_add(
                        out=dst[:, ib, sc * 512 : sc * 512 + 512],
                        in0=ps,
                        scalar1=b_sb[:, ib : ib + 1],
                    )

            def project_v(sc):
                """One 512-seq chunk of the v projection into vh (bf16)."""
                xts = load_and_transpose(v_h, sc)
                for sb in range(4):
                    ps = ppsum.tile([128, 512], F32, tag="p")
                    for ec in range(EC):
                        nc.tensor.matmul(
                            ps,
                            xts[ec][:, sb * 128 : sb * 128 + 128],
                            wv_sb[:, ec, :],
                            start=(ec == 0),
                            stop=(ec == EC - 1),
                        )
                    kb = sc * 4 + sb
                    nc.vector.tensor_copy(
                        out=vh[:, kb, :, 0:64],
                        in_=ps.rearrange("p (h d) -> p h d", h=HL),
                    )

            # k and v first so attention can start as soon as the first q
            # chunk lands; v-projection bias is applied post-normalization
            # on attnT.
            for sc in range(nk // 512):
                project_qk(khT, k_h, wk_sb, bk_sb, sc)
            for sc in range(nk // 512):
                project_v(sc)
            for sc in range(nq // 512):
                project_qk(qhT, q_h, wq_sb, bq_sb, sc)

        # -------- Phase C+D: attention with interleaved output projection ---
        with ExitStack() as phc:
            cd_pool = phc.enter_context(tc.tile_pool(name="cd", bufs=1))
            wo_sb = cd_pool.tile([128, IB, E], F32R)
            nc.sync.dma_start(
                out=wo_sb, in_=wo_h.ap().rearrange("(c p) e -> p c e", p=128)
            )
            attnT = cd_pool.tile([128, IB, nq], F32R)

            exps_p = phc.enter_context(tc.tile_pool(name="exps", bufs=exps_bufs))
            rl_p = phc.enter_context(tc.tile_pool(name="rl", bufs=4))
            rld_p = phc.enter_context(
                tc.tile_pool(name="rld", bufs=8, space="DRAM")
            )
            out_p = phc.enter_context(tc.tile_pool(name="outsb", bufs=4))
            spsum = phc.enter_context(
                tc.tile_pool(name="spsum", bufs=sp_bufs, space="PSUM")
            )
            avpsum = phc.enter_context(
                tc.tile_pool(name="avpsum", bufs=av_bufs, space="PSUM")
            )
            opsum = phc.enter_context(
                tc.tile_pool(name="opsum", bufs=op_bufs, space="PSUM")
            )

            for jq in range(nq // 512):
                q0 = jq * 512
                for p_ in range(IB):
                    # QK^T for both heads of the pair: h2=0 scores in psum
                    # cols 0:512 (bank A), h2=1 in cols 512:1024 (bank B) -
                    # independent 64x128 row-tiles that overlap on the PE.
                    exp_tiles = []
                    for kb in range(KB):
                        ps = spsum.tile([128, 1024], F32, tag="s")
                        for h2 in range(2):
                            hp = slice(h2 * 64, h2 * 64 + 64)
                            nc.tensor.matmul(
                                ps[:, h2 * 512 : h2 * 512 + 512],
                                khT[hp, p_, kb * 128 : kb * 128 + 128],
                                qhT[hp, p_, q0 : q0 + 512],
                                start=True,
                                stop=True,
                            )
                        et = exps_p.tile([128, 1024], BF16, tag="e")
                        nc.scalar.activation(
                            out=et,
                            in_=ps,
                            func=mybir.ActivationFunctionType.Exp,
                            scale=SCALE,
                        )
                        exp_tiles.append(et)
                    # AV (ones-augmented): rows 0..63 data, row 64 = denom
                    for h2 in range(2):
                        h = 2 * p_ + h2
                        ps = avpsum.tile([65, 512], F32, tag="av")
                        for kb in range(KB):
                            nc.tensor.matmul(
                                ps,
                                vh[:, kb, h, :],
                                exp_tiles[kb][:, h2 * 512 : h2 * 512 + 512],
                                start=(kb == 0),
                                stop=(kb == KB - 1),
                            )
                        # 1/l: reciprocal of the denom row, bounce through
                        # DRAM, DMA back with a stride-0 partition AP
                        rl1 = rl_p.tile([1, 512], F32, tag="rl1")
                        nc.vector.reciprocal(out=rl1, in_=ps[64:65, :])
                        rld = rld_p.tile([1, 512], F32, tag="rld")
                        nc.sync.dma_start(out=rld, in_=rl1)
                        rlb = rl_p.tile([64, 512], F32, tag="rlb")
                        nc.sync.dma_start(
                            out=rlb,
                            in_=bass.AP(
                                tensor=rld.tensor,
                                offset=rld.offset,
                                ap=[[0, 64]] + list(rld.ap)[1:],
                            ),
                        )
                        hrows = slice(0, 64) if h2 == 0 else slice(64, 128)
                        nc.vector.tensor_mul(
                            attnT[hrows, p_, q0 : q0 + 512],
                            ps[0:64, :],
                            rlb,
                        )
                    # + bv (deferred v-projection bias: P @ (vh + bv) =
                    #   P@vh + l*bv, so attnT just gains bv after normalize)
                    nc.vector.tensor_scalar_add(
                        out=attnT[:, p_, q0 : q0 + 512],
                        in0=attnT[:, p_, q0 : q0 + 512],
                        scalar1=bv_sb[:, p_ : p_ + 1],
                    )

                # output projection for this 512-q chunk
                for m2 in range(4):
                    m = jq * 4 + m2
                    for half in range(2):
                        ps = opsum.tile([128, 512], F32, tag="o")
                        for ic in range(IB):
                            nc.tensor.matmul(
                                ps,
                                attnT[:, ic, m * 128 : m * 128 + 128],
                                wo_sb[:, ic, half * 512 : half * 512 + 512],
                                start=(ic == 0),
                                stop=(ic == IB - 1),
                            )
                        osb = out_p.tile([128, 512], F32R, tag="osb")
                        nc.vector.tensor_copy(out=osb, in_=ps)
                        nc.sync.dma_start(
                            out=out_h.ap()[
                                m * 128 : m * 128 + 128,
                                half * 512 : half * 512 + 512,
                            ],
                            in_=osb,
                        )

    if split_waits:
        _split_waits(nc)
    return nc


_CACHED = {}


def _get_nc(nq=NQ, nk=NK):
    key = (nq, nk)
    if key not in _CACHED:
        _CACHED[key] = build_nc(nq, nk)
    return _CACHED[key]


def shard_inputs(q, k, v, wq, bq, wk, bk, wv, bv, wo):
    """8 per-core input maps: core c -> (batch c//2, head-group c%2)."""
    in_maps = []
    for c in range(N_CORES):
        b, g = c // 2, c % 2
        sl = slice(g * IL, (g + 1) * IL)
        in_maps.append(
            {
                "q": np.ascontiguousarray(q[b]),
                "k": np.ascontiguousarray(k[b]),
                "v": np.ascontiguousarray(v[b]),
                "wq": np.ascontiguousarray(wq[:, sl]),
                "wk": np.ascontiguousarray(wk[:, sl]),
                "wv": np.ascontiguousarray(wv[:, sl]),
                "bq": np.ascontiguousarray(bq[sl]),
                "bk": np.ascontiguousarray(bk[sl]),
                "bv": np.ascontiguousarray(bv[sl]),
                "wo": np.ascontiguousarray(wo[sl, :]),
                "ident": np.eye(128, dtype=np.float32),
            }
        )
    return in_maps


def kernel(q, k, v, wq, bq, wk, bk, wv, bv, wo, bo, _trace=False):
    from concourse.bass_utils import run_bass_kernel_spmd

    q, k, v = (np.asarray(x, np.float32) for x in (q, k, v))
    wq, bq, wk, bk, wv, bv, wo, bo = (
        np.asarray(x, np.float32) for x in (wq, bq, wk, bk, wv, bv, wo, bo)
    )
    nc = _get_nc()
    in_maps = shard_inputs(q, k, v, wq, bq, wk, bk, wv, bv, wo)
    try:
        res = run_bass_kernel_spmd(
            nc, in_maps, core_ids=list(range(N_CORES)), trace=_trace
        )
    except Exception:
        if not _trace:
            raise
        import traceback

        traceback.print_exc()
        print("trace run failed; retrying without trace", flush=True)
        res = run_bass_kernel_spmd(
            nc, in_maps, core_ids=list(range(N_CORES)), trace=False
        )
    parts = np.stack([res.results[c]["out"] for c in range(N_CORES)])
    out = parts.reshape(B, 2, NQ, E).sum(axis=1) + bo[None, None, :]
    if _trace:
        kernel.last_results = res
    return out.astype(np.float32)
